# revision 11
# baseline (speedup 1.0000x reference)
"""Trainium2 Bass kernel for Transformer-XL style relative-position MHSA.

Problem: nn_MultiHeadSelfAttention_14989435863450
  B=2, S=2048, D=512, H=8, dh=64, fp32 I/O.

Sharding (8 cores): core c -> batch b = c//4, head pair h0 = 2*(c%4).
Each core computes its 2 heads' attention and the partial output
projection (out_slice @ Wo[slice]); host sums 4 partials per batch and
adds the constant (bv @ Wo + bo) row vector.

Math folds (exact):
  - bq folds into u,v:  u_eff = (u + bq) / sqrt(D)
  - bk adds a per-query-row constant to scores -> cancels in softmax
  - bv contributes attn-weighted 1 * bv = bv -> host-side constant
  - 1/sqrt(D) folded into q at evacuation time

Design (v1, transposed-attention orientation):
  - All matmul operands bf16 (fp32r streams at ~2 cyc/row; bf16 at 1).
  - Scores are built TRANSPOSED: sT[k, q] = kT.T @ qTu per 128-k tile,
    so attn never needs a PE transpose before the attn@v contraction.
  - Rel-shift via DRAM buffer PB[S, S+1] (rows [0 | posrow_i]); the
    shifted [q, k] view (flat[S + q*S + k]) is read back with the XBAR
    DMA transpose -> sposT[k, q] lands directly in the transposed
    orientation at near line rate on the DMA engines.
  - Softmax denominator rides as a ones-column in the V matrix:
    po[65, q] = [vv | 1].T @ expT; row 64 is Z. Normalization:
    rec = 1/Z (DVE), PE-broadcast (ones[1,64].T @ rec), o2 = po * bcast.
  - Output proj per 128-q block: pw[q, D] = sum_h o2_h.T @ Wo_h.
"""

import math
from contextlib import ExitStack

import numpy as np
import ml_dtypes

import concourse.bass as bass
import concourse.bacc as bacc_mod
import concourse.mybir as mybir
import concourse.tile as tile
from concourse.bass import ts, ds
from concourse.bass_utils import run_bass_kernel_spmd

FP32 = mybir.dt.float32
F32R = mybir.dt.float32r
BF16 = mybir.dt.bfloat16

D_MODEL = 512
NUM_HEADS = 8
D_HEAD = 64
DH2 = 2 * D_HEAD  # head-pair width per core
B_FULL = 2
S_FULL = 2048
P = 128
CH = 512          # q-chunk / score column chunk
ISQ = 1.0 / math.sqrt(D_MODEL)

Exp = mybir.ActivationFunctionType.Exp
ADD = mybir.AluOpType.add
MULT = mybir.AluOpType.mult


def build_nc(S=S_FULL):
    """Build the single-core Bass program (SPMD: same program, 8 cores)."""
    nc = bacc_mod.Bacc()
    NB = S // P            # 128-row q blocks (16)
    NK = S // P            # 128-row k tiles (16)
    NCH = S // CH          # 512-wide chunks (4)
    KD = D_MODEL // P      # contraction tiles over D (4)
    NG = NK // 2           # 2-kt groups per (h, chunk) (8)

    xT = nc.declare_dram_parameter("xT", [D_MODEL, S], BF16, isOutput=False)
    posT = nc.declare_dram_parameter("posT", [D_MODEL, S], BF16, isOutput=False)
    Wq = nc.declare_dram_parameter("Wq", [D_MODEL, DH2], BF16, isOutput=False)
    Wk = nc.declare_dram_parameter("Wk", [D_MODEL, DH2], BF16, isOutput=False)
    Wv = nc.declare_dram_parameter("Wv", [D_MODEL, DH2], BF16, isOutput=False)
    Wp = nc.declare_dram_parameter("Wp", [D_MODEL, DH2], BF16, isOutput=False)
    Wo = nc.declare_dram_parameter("Wo", [DH2, D_MODEL], BF16, isOutput=False)
    ueff = nc.declare_dram_parameter("ueff", [DH2, 1], FP32, isOutput=False)
    veff = nc.declare_dram_parameter("veff", [DH2, 1], FP32, isOutput=False)
    out_partial = nc.declare_dram_parameter("out_partial", [S, D_MODEL], FP32, isOutput=True)

    with ExitStack() as ctx:
        tc = ctx.enter_context(tile.TileContext(nc))
        consts = ctx.enter_context(tc.tile_pool(name="consts", bufs=1))
        blk = ctx.enter_context(tc.tile_pool(name="blk", bufs=3))
        epool = ctx.enter_context(tc.tile_pool(name="epool", bufs=2))
        dram = ctx.enter_context(tc.tile_pool(name="dram", bufs=1, space="DRAM"))
        # PSUM budget (8 banks): psC 2x[128,2,512] (4) + psPos 1x[128,2,512] (2)
        # + psPo 2x[65,512] (2, po/bcast rotate)
        psC = ctx.enter_context(tc.tile_pool(name="psC", bufs=2, space="PSUM"))
        psPos = ctx.enter_context(tc.tile_pool(name="psPos", bufs=1, space="PSUM"))
        psPo = ctx.enter_context(tc.tile_pool(name="psPo", bufs=2, space="PSUM"))

        # ---- load constants / inputs ----
        xT_sb = consts.tile([P, KD, S], BF16)
        nc.sync.dma_start(xT_sb[:], xT.rearrange("(o p) s -> p o s", p=P))
        posT_sb = consts.tile([P, KD, S], BF16)
        nc.sync.dma_start(posT_sb[:], posT.rearrange("(o p) s -> p o s", p=P))
        w_sbs = {}
        for nm, handle in (("Wq", Wq), ("Wk", Wk), ("Wv", Wv), ("Wp", Wp)):
            w_sb = consts.tile([P, KD, DH2], BF16, name=f"{nm}_sb")
            nc.sync.dma_start(w_sb[:], handle.rearrange("(o p) m -> p o m", p=P))
            w_sbs[nm] = w_sb
        Wo_sb = consts.tile([D_HEAD, 2, D_MODEL], BF16)
        nc.sync.dma_start(Wo_sb[:], Wo.rearrange("(h d) n -> d h n", h=2))
        ueff_sb = consts.tile([DH2, 1], FP32)
        nc.sync.dma_start(ueff_sb[:], ueff[:, :])
        veff_sb = consts.tile([DH2, 1], FP32)
        nc.sync.dma_start(veff_sb[:], veff[:, :])
        # ones row lives at partition 64 to match po's Z row (DVE lanes are
        # partition-locked, so all row-64 plumbing stays on partition 64)
        ones_sb = consts.tile([D_HEAD + 1, D_HEAD], F32R)
        ones_f32 = consts.tile([D_HEAD + 1, D_HEAD], FP32)
        nc.vector.memset(ones_f32[D_HEAD : D_HEAD + 1, :], 1.0)
        nc.vector.tensor_copy(
            ones_sb[D_HEAD : D_HEAD + 1, :], ones_f32[D_HEAD : D_HEAD + 1, :]
        )

        # ---- projections (bf16 results in SBUF) ----
        qTu = consts.tile([DH2, S], BF16)
        qTv = consts.tile([DH2, S], BF16)
        kT = consts.tile([DH2, S], BF16)
        pT = consts.tile([DH2, S], BF16)

        def proj_groups(w_sb, src_sb, evac):
            # two [128, 2, 512] psum groups per projection
            for g in range(NCH // 2):
                pg = psC.tile([P, 2, CH], FP32, tag="ps", name="pg")
                for j in range(2):
                    chn = 2 * g + j
                    for kt in range(KD):
                        nc.tensor.matmul(
                            pg[:, j, :],
                            lhsT=w_sb[:, kt, :],
                            rhs=src_sb[:, kt, ts(chn, CH)],
                            start=(kt == 0),
                            stop=(kt == KD - 1),
                        )
                evac(g, pg)

        def evac_q(g, pg):
            sl = ds(g * 2 * CH, 2 * CH)
            pv = pg[:].rearrange("p a b -> p (a b)")
            nc.vector.tensor_scalar(qTu[:, sl], pv, ISQ, ueff_sb[:, 0:1], MULT, ADD)
            nc.vector.tensor_scalar(qTv[:, sl], pv, ISQ, veff_sb[:, 0:1], MULT, ADD)

        def evac_to(dst):
            def evac(g, pg):
                sl = ds(g * 2 * CH, 2 * CH)
                nc.scalar.copy(dst[:, sl], pg[:].rearrange("p a b -> p (a b)"))
            return evac

        proj_groups(w_sbs["Wq"], xT_sb, evac_q)
        proj_groups(w_sbs["Wk"], xT_sb, evac_to(kT))
        proj_groups(w_sbs["Wp"], posT_sb, evac_to(pT))

        # v natural [k, dh] + ones column -> vv_aug [128, NK, 2, 65]
        vv_aug = consts.tile([P, NK, 2, D_HEAD + 1], BF16)
        nc.vector.memset(vv_aug[:, :, :, D_HEAD : D_HEAD + 1], 1.0)
        for sg in range(NK // 2):
            pv = psPos.tile([P, 2, CH], FP32, tag="pos", name="pv")
            for j in range(2):
                st = 2 * sg + j
                for kt in range(KD):
                    nc.tensor.matmul(
                        pv[:, j, 0:DH2],
                        lhsT=xT_sb[:, kt, ts(st, P)],
                        rhs=w_sbs["Wv"][:, kt, :],
                        start=(kt == 0),
                        stop=(kt == KD - 1),
                    )
            for j in range(2):
                src = pv[:, j, 0:DH2].rearrange("p (h d) -> p h d", h=2)
                nc.vector.tensor_copy(vv_aug[:, 2 * sg + j, :, 0:D_HEAD], src)

        # ---- per-head DRAM pos-score buffers (padded for the rel-shift) ----
        PB = [dram.tile([S, S + 1], BF16, name=f"pb{h}") for h in range(2)]

        def pos_block(h, ib):
            """pos scores (orientation A) for q rows [128*ib, +128) -> PB[h]."""
            pe = blk.tile([P, S + 1], BF16, tag="posext", name="pe")
            nc.vector.memset(pe[:, 0:1], 0.0)
            for g in range(NCH // 2):
                pp = psPos.tile([P, 2, CH], FP32, tag="pos", name="pp")
                for j in range(2):
                    chn = 2 * g + j
                    nc.tensor.matmul(
                        pp[:, j, :],
                        lhsT=qTv[ds(h * D_HEAD, D_HEAD), ts(ib, P)],
                        rhs=pT[ds(h * D_HEAD, D_HEAD), ts(chn, CH)],
                        start=True,
                        stop=True,
                    )
                dst = pe[:, ds(1 + g * 2 * CH, 2 * CH)]
                src = pp[:].rearrange("p a b -> p (a b)")
                if g % 2 == 0:
                    nc.scalar.copy(dst, src)
                else:
                    nc.vector.tensor_copy(dst, src)
            nc.scalar.dma_start(PB[h][ts(ib, P), :], pe[:])

        def content_group(h, c, g, expT):
            """2 k-tiles of transposed scores for q-chunk c -> expT."""
            kt0 = 2 * g
            sp = blk.tile([P, 2, CH], BF16, tag="spos", name="sp")
            flat = PB[h].flatten()
            qview = flat[ds(S + c * CH * S, CH * S)].rearrange("(q k) -> q k", k=S)
            for j in range(2):
                nc.sync.dma_start(
                    sp[:, j, :], qview[:, ts(kt0 + j, P)], transpose=True
                )
            ps = psC.tile([P, 2, CH], FP32, tag="ps", name="ps")
            for j in range(2):
                nc.tensor.matmul(
                    ps[:, j, :],
                    lhsT=kT[ds(h * D_HEAD, D_HEAD), ts(kt0 + j, P)],
                    rhs=qTu[ds(h * D_HEAD, D_HEAD), ts(c, CH)],
                    start=True,
                    stop=True,
                )
            sc = blk.tile([P, 2, CH], BF16, tag="sc", name="sc")
            nc.vector.tensor_tensor(sc[:], ps[:], sp[:], ADD)
            nc.scalar.activation(expT[:, ts(g, 2), :], sc[:], Exp)

        def head_chunk(h, c, interleave):
            """scores + softmax-exp + attn@v for q-chunk c, head h."""
            expT = epool.tile([P, NK, CH], BF16, tag="expT", name="expT")
            inter = list(interleave)
            for g in range(NG):
                content_group(h, c, g, expT)
                if inter:
                    inter.pop(0)()
            for fn in inter:
                fn()
            po = psPo.tile([D_HEAD + 1, CH], FP32, tag="po", name="po")
            for kt in range(NK):
                nc.tensor.matmul(
                    po[:],
                    lhsT=vv_aug[:, kt, h, :],
                    rhs=expT[:, kt, :],
                    start=(kt == 0),
                    stop=(kt == NK - 1),
                )
            rec = blk.tile([D_HEAD + 1, CH], F32R, tag="rec", name="rec")
            with nc.allow_low_precision(reason="f32r is bitwise fp32"):
                nc.vector.reciprocal(
                    rec[D_HEAD : D_HEAD + 1, :], po[D_HEAD : D_HEAD + 1, :]
                )
            bc = psPo.tile([D_HEAD + 1, CH], FP32, tag="po", name="bc")
            nc.tensor.matmul(
                bc[0:D_HEAD, :],
                lhsT=ones_sb[D_HEAD : D_HEAD + 1, :],
                rhs=rec[D_HEAD : D_HEAD + 1, :],
                start=True,
                stop=True,
            )
            bcs = blk.tile([D_HEAD, CH], F32R, tag="bcs", name="bcs")
            if h == 0:
                nc.scalar.copy(bcs[:], bc[0:D_HEAD, :])
            else:
                nc.vector.tensor_copy(bcs[:], bc[0:D_HEAD, :])
            o2 = o2s[(h, c)]
            nc.vector.tensor_tensor(o2[:], po[0:D_HEAD, :], bcs[:], MULT)

        o2s = {}

        def pw_block(c, j):
            """output projection for q block ib = 4c + j."""
            ib = NCH * c + j
            pw = psC.tile([P, 2, CH], FP32, tag="ps", name="pw")
            for h in range(2):
                nc.tensor.matmul(
                    pw[:, 0, :],
                    lhsT=o2s[(h, c)][:, ts(j, P)],
                    rhs=Wo_sb[:, h, :],
                    start=(h == 0),
                    stop=(h == 1),
                )
            fin = blk.tile([P, D_MODEL], FP32, tag="fin", name="fin")
            nc.vector.tensor_copy(fin[:], pw[:, 0, :])
            nc.scalar.dma_start(out_partial[ts(ib, P), :], fin[:])

        # ---- main pipeline ----
        for h in range(2):
            for c in range(NCH):
                o2s[(h, c)] = blk.tile(
                    [D_HEAD, CH], BF16, tag=f"o2_{h}_{c}", name="o2"
                )

        # prologue: pos blocks 0..4 for h0
        for ib in range(5):
            pos_block(0, ib)

        for c in range(NCH):
            # h0 chunk; interleave: pw for chunk c-1, then pos(h1) lookahead
            inter0 = []
            if c > 0:
                inter0 += [
                    (lambda cc, jj: (lambda: pw_block(cc, jj)))(c - 1, j)
                    for j in range(NCH)
                ]
            if c == 0:
                h1_blocks = list(range(5))
            else:
                h1_blocks = [b for b in range(4 * c + 1, 4 * c + 5) if b < NB]
            inter0 += [
                (lambda bb: (lambda: pos_block(1, bb)))(b) for b in h1_blocks
            ]
            head_chunk(0, c, inter0)
            # h1 chunk; interleave: pos(h0) lookahead for chunk c+1
            h0_blocks = [b for b in range(4 * c + 5, 4 * c + 9) if b < NB]
            inter1 = [(lambda bb: (lambda: pos_block(0, bb)))(b) for b in h0_blocks]
            head_chunk(1, c, inter1)

        for j in range(NCH):
            pw_block(NCH - 1, j)

    nc.finalize()
    return nc


# ---------------- host side ----------------

_NC_CACHE = {}


def _get_nc(S=S_FULL):
    if S not in _NC_CACHE:
        _NC_CACHE[S] = build_nc(S)
    return _NC_CACHE[S]


def make_in_maps(inputs, S=S_FULL, n_cores=8):
    bf16 = ml_dtypes.bfloat16
    x = np.asarray(inputs["x"], np.float32)
    pos = np.asarray(inputs["pos_embedding"], np.float32)
    Wq = np.asarray(inputs["Wq"], np.float32)
    bq = np.asarray(inputs["bq"], np.float32)
    Wk = np.asarray(inputs["Wk"], np.float32)
    Wv = np.asarray(inputs["Wv"], np.float32)
    Wp = np.asarray(inputs["Wp"], np.float32)
    u = np.asarray(inputs["u"], np.float32)
    v = np.asarray(inputs["v"], np.float32)
    Wo = np.asarray(inputs["Wo"], np.float32)

    xTb = [np.ascontiguousarray(x[b, :S].T).astype(bf16) for b in range(B_FULL)]
    posTb = [np.ascontiguousarray(pos[b, :S].T).astype(bf16) for b in range(B_FULL)]

    in_maps = []
    for c in range(n_cores):
        b = c // 4
        h0 = 2 * (c % 4)
        sl = slice(h0 * D_HEAD, (h0 + 2) * D_HEAD)
        u_eff = ((u[h0 : h0 + 2].reshape(-1) + bq[sl]) * ISQ).astype(np.float32)
        v_eff = ((v[h0 : h0 + 2].reshape(-1) + bq[sl]) * ISQ).astype(np.float32)
        in_maps.append(
            {
                "xT": xTb[b],
                "posT": posTb[b],
                "Wq": np.ascontiguousarray(Wq[:, sl]).astype(bf16),
                "Wk": np.ascontiguousarray(Wk[:, sl]).astype(bf16),
                "Wv": np.ascontiguousarray(Wv[:, sl]).astype(bf16),
                "Wp": np.ascontiguousarray(Wp[:, sl]).astype(bf16),
                "Wo": np.ascontiguousarray(Wo[sl, :]).astype(bf16),
                "ueff": u_eff.reshape(DH2, 1),
                "veff": v_eff.reshape(DH2, 1),
            }
        )
    return in_maps


def assemble(inputs, results, S=S_FULL):
    bv = np.asarray(inputs["bv"], np.float64)
    Wo = np.asarray(inputs["Wo"], np.float64)
    bo = np.asarray(inputs["bo"], np.float64)
    const = (bv @ Wo + bo).astype(np.float32)
    out = np.zeros((B_FULL, S, D_MODEL), np.float32)
    for c, res in enumerate(results):
        out[c // 4] += res["out_partial"]
    out += const[None, None, :]
    return out


def _run(inputs, trace=False, **kw):
    nc = _get_nc(S_FULL)
    in_maps = make_in_maps(inputs, S_FULL)
    res = run_bass_kernel_spmd(nc, in_maps, list(range(8)), trace=trace, **kw)
    out = assemble(inputs, res.results, S_FULL)
    return out, res


def kernel(**inputs) -> np.ndarray:
    out, _ = _run(inputs, trace=False)
    return out


# revision 12
# speedup vs baseline: 1.2029x; 1.2029x over previous
"""Trainium2 Bass kernel for Transformer-XL style relative-position MHSA.

Problem: nn_MultiHeadSelfAttention_14989435863450
  B=2, S=2048, D=512, H=8, dh=64, fp32 I/O.

Sharding (8 cores): core c -> batch b = c//4, head pair h0 = 2*(c%4).
Each core computes its 2 heads' attention and the partial output
projection; host sums 4 partials per batch and adds (bv @ Wo + bo).

Math folds (exact):
  - bq folds into u,v:  u_eff = (u + bq) / sqrt(D)
  - bk adds a per-query-row constant to scores -> cancels in softmax
  - bv contributes attn-weighted 1 * bv = bv -> host-side constant
  - 1/sqrt(D) folded into q at evacuation time

Design (v2, transposed attention, kt-outer):
  - All matmul operands bf16.
  - Scores built TRANSPOSED: sT[k, q] = kT.T @ qTu per 128-k tile; the
    attn matrix never needs a PE transpose before the attn@v matmul.
  - Rel-shift via DRAM buffer PB[S, S+1]; the shifted [q, k] view
    (flat[S + q*S + k]) is read back TRANSPOSED by the XBAR DMA in
    [1024, 128] panels (one per (h, q-half, k-tile)), alternating the
    two HWDGE rings (sync / scalar) so transposes pipeline.
  - Loop order kt-outer within a q-half: po accumulates into a 4-bank
    PSUM tile [65, 4*512] (slice per 512-q chunk); the vv stationary is
    reused across the half's two chunks.
  - Softmax denominator rides as a ones-column in vv: po row 64 is Z.
    Normalize per chunk: bcast = ones.T @ Z (PE), rec = 1/bcast (DVE,
    full-lane), o2 = po * rec.
  - PB writes and output writes ride the GPSIMD SWDGE ring, keeping the
    sync/scalar HWDGE rings free for the transposed reads.
"""

import math
from contextlib import ExitStack

import numpy as np
import ml_dtypes

import concourse.bass as bass
import concourse.bacc as bacc_mod
import concourse.mybir as mybir
import concourse.tile as tile
from concourse.bass import ts, ds
from concourse.bass_utils import run_bass_kernel_spmd

FP32 = mybir.dt.float32
F32R = mybir.dt.float32r
BF16 = mybir.dt.bfloat16

D_MODEL = 512
NUM_HEADS = 8
D_HEAD = 64
DH2 = 2 * D_HEAD
B_FULL = 2
S_FULL = 2048
P = 128
CH = 512
ISQ = 1.0 / math.sqrt(D_MODEL)

Exp = mybir.ActivationFunctionType.Exp
ADD = mybir.AluOpType.add
MULT = mybir.AluOpType.mult


def build_nc(S=S_FULL):
    nc = bacc_mod.Bacc()
    NB = S // P        # 16 q blocks
    NK = S // P        # 16 k tiles
    NCH = S // CH      # 4 chunks
    KD = D_MODEL // P  # 4
    HALF = S // 2      # 1024

    xT = nc.declare_dram_parameter("xT", [D_MODEL, S], BF16, isOutput=False)
    posT = nc.declare_dram_parameter("posT", [D_MODEL, S], BF16, isOutput=False)
    Wq = nc.declare_dram_parameter("Wq", [D_MODEL, DH2], BF16, isOutput=False)
    Wk = nc.declare_dram_parameter("Wk", [D_MODEL, DH2], BF16, isOutput=False)
    Wv = nc.declare_dram_parameter("Wv", [D_MODEL, DH2], BF16, isOutput=False)
    Wp = nc.declare_dram_parameter("Wp", [D_MODEL, DH2], BF16, isOutput=False)
    Wo = nc.declare_dram_parameter("Wo", [DH2, D_MODEL], BF16, isOutput=False)
    ueff = nc.declare_dram_parameter("ueff", [DH2, 1], FP32, isOutput=False)
    veff = nc.declare_dram_parameter("veff", [DH2, 1], FP32, isOutput=False)
    out_partial = nc.declare_dram_parameter("out_partial", [S, D_MODEL], FP32, isOutput=True)

    with ExitStack() as ctx:
        tc = ctx.enter_context(tile.TileContext(nc))
        consts = ctx.enter_context(tc.tile_pool(name="consts", bufs=1))
        blk = ctx.enter_context(tc.tile_pool(name="blk", bufs=3))
        spool = ctx.enter_context(tc.tile_pool(name="spool", bufs=5))
        dram = ctx.enter_context(tc.tile_pool(name="dram", bufs=1, space="DRAM"))
        # PSUM (8 banks): psAcc 1x[65,4,512] (4) + psC 2x[128,2,512] (4)
        psAcc = ctx.enter_context(tc.tile_pool(name="psAcc", bufs=1, space="PSUM"))
        psC = ctx.enter_context(tc.tile_pool(name="psC", bufs=2, space="PSUM"))

        # ---- load constants / inputs ----
        xT_sb = consts.tile([P, KD, S], BF16)
        nc.sync.dma_start(xT_sb[:], xT.rearrange("(o p) s -> p o s", p=P))
        posT_sb = consts.tile([P, KD, S], BF16)
        nc.sync.dma_start(posT_sb[:], posT.rearrange("(o p) s -> p o s", p=P))
        w_sbs = {}
        for nm, handle in (("Wq", Wq), ("Wp", Wp), ("Wk", Wk), ("Wv", Wv)):
            w_sb = consts.tile([P, KD, DH2], BF16, name=f"{nm}_sb")
            nc.sync.dma_start(w_sb[:], handle.rearrange("(o p) m -> p o m", p=P))
            w_sbs[nm] = w_sb
        Wo_sb = consts.tile([D_HEAD, 2, D_MODEL], BF16)
        nc.sync.dma_start(Wo_sb[:], Wo.rearrange("(h d) n -> d h n", h=2))
        ueff_sb = consts.tile([DH2, 1], FP32)
        nc.sync.dma_start(ueff_sb[:], ueff[:, :])
        veff_sb = consts.tile([DH2, 1], FP32)
        nc.sync.dma_start(veff_sb[:], veff[:, :])
        # ones row on partition 64 (matches po's Z row; DVE lanes are
        # partition-locked so all row-64 plumbing stays on partition 64)
        ones_sb = consts.tile([D_HEAD + 1, D_HEAD], F32R)
        ones_f32 = consts.tile([D_HEAD + 1, D_HEAD], FP32)
        nc.vector.memset(ones_f32[D_HEAD : D_HEAD + 1, :], 1.0)
        nc.vector.tensor_copy(
            ones_sb[D_HEAD : D_HEAD + 1, :], ones_f32[D_HEAD : D_HEAD + 1, :]
        )

        qTu = consts.tile([DH2, S], BF16)
        qTv = consts.tile([DH2, S], BF16)
        kT = consts.tile([DH2, S], BF16)
        pT = consts.tile([DH2, S], BF16)
        vv_aug = consts.tile([P, NK, 2, D_HEAD + 1], BF16)
        nc.vector.memset(vv_aug[:, :, :, D_HEAD : D_HEAD + 1], 1.0)

        def proj_groups(w_sb, src_sb, evac):
            for g in range(NCH // 2):
                pg = psC.tile([P, 2, CH], FP32, tag="ps", name="pg")
                for j in range(2):
                    chn = 2 * g + j
                    for kt in range(KD):
                        nc.tensor.matmul(
                            pg[:, j, :],
                            lhsT=w_sb[:, kt, :],
                            rhs=src_sb[:, kt, ts(chn, CH)],
                            start=(kt == 0),
                            stop=(kt == KD - 1),
                        )
                evac(g, pg)

        def evac_q(g, pg):
            sl = ds(g * 2 * CH, 2 * CH)
            pv = pg[:].rearrange("p a b -> p (a b)")
            nc.vector.tensor_scalar(qTu[:, sl], pv, ISQ, ueff_sb[:, 0:1], MULT, ADD)
            nc.vector.tensor_scalar(qTv[:, sl], pv, ISQ, veff_sb[:, 0:1], MULT, ADD)

        def evac_to(dst):
            def evac(g, pg):
                sl = ds(g * 2 * CH, 2 * CH)
                nc.scalar.copy(dst[:, sl], pg[:].rearrange("p a b -> p (a b)"))
            return evac

        def proj_v():
            for sg in range(NK // 2):
                pv = psC.tile([P, 2, CH], FP32, tag="ps", name="pv")
                for j in range(2):
                    st = 2 * sg + j
                    for kt in range(KD):
                        nc.tensor.matmul(
                            pv[:, j, 0:DH2],
                            lhsT=xT_sb[:, kt, ts(st, P)],
                            rhs=w_sbs["Wv"][:, kt, :],
                            start=(kt == 0),
                            stop=(kt == KD - 1),
                        )
                for j in range(2):
                    src = pv[:, j, 0:DH2].rearrange("p (h d) -> p h d", h=2)
                    nc.vector.tensor_copy(vv_aug[:, 2 * sg + j, :, 0:D_HEAD], src)

        PB = [dram.tile([S, S + 1], BF16, name=f"pb{h}") for h in range(2)]

        def pos_block(h, ib):
            """pos scores (orientation A) for q rows [128*ib, +128) -> PB[h]."""
            pe = blk.tile([P, S + 1], BF16, tag="posext", name="pe")
            nc.vector.memset(pe[:, 0:1], 0.0)
            for g in range(NCH // 2):
                pp = psC.tile([P, 2, CH], FP32, tag="ps", name="pp")
                for j in range(2):
                    chn = 2 * g + j
                    nc.tensor.matmul(
                        pp[:, j, :],
                        lhsT=qTv[ds(h * D_HEAD, D_HEAD), ts(ib, P)],
                        rhs=pT[ds(h * D_HEAD, D_HEAD), ts(chn, CH)],
                        start=True,
                        stop=True,
                    )
                dst = pe[:, ds(1 + g * 2 * CH, 2 * CH)]
                src = pp[:].rearrange("p a b -> p (a b)")
                if (ib + g) % 2 == 0:
                    nc.scalar.copy(dst, src)
                else:
                    nc.vector.tensor_copy(dst, src)
            nc.gpsimd.dma_start(PB[h][ts(ib, P), :], pe[:])

        o2s = {}
        for h in range(2):
            o2s[h] = blk.tile([D_HEAD, NCH, CH], BF16, tag=f"o2_{h}", name="o2")

        def kt_step(h, half, kt, po, interleave):
            """one k-tile for q rows [half*1024, +1024): read spT, scores,
            exp, accumulate attn@v into po."""
            sp = spool.tile([P, 2, CH], BF16, tag="spos", name="sp")
            flat = PB[h].flatten()
            qview = flat[ds(S + half * HALF * S, HALF * S)].rearrange(
                "(q k) -> q k", k=S
            )
            eng = nc.sync if kt % 2 == 0 else nc.scalar
            eng.dma_start(sp[:].rearrange("p a b -> p (a b)"),
                          qview[:, ts(kt, P)], transpose=True)
            ps = psC.tile([P, 2, CH], FP32, tag="ps", name="ps")
            for j in range(2):
                c = 2 * half + j
                nc.tensor.matmul(
                    ps[:, j, :],
                    lhsT=kT[ds(h * D_HEAD, D_HEAD), ts(kt, P)],
                    rhs=qTu[ds(h * D_HEAD, D_HEAD), ts(c, CH)],
                    start=True,
                    stop=True,
                )
            if interleave:
                interleave.pop(0)()
            sc = blk.tile([P, 2, CH], BF16, tag="sc", name="sc")
            nc.vector.tensor_tensor(sc[:], ps[:], sp[:], ADD)
            et = blk.tile([P, 2, CH], BF16, tag="et", name="et")
            nc.scalar.activation(et[:], sc[:], Exp)
            for j in range(2):
                c = 2 * half + j
                nc.tensor.matmul(
                    po[:, c, :],
                    lhsT=vv_aug[:, kt, h, :],
                    rhs=et[:, j, :],
                    start=(kt == 0),
                    stop=(kt == NK - 1),
                )

        def norm_head(h, po):
            """o2 = po[0:64] / Z per chunk (Z = po row 64)."""
            zrow = blk.tile([D_HEAD + 1, S], F32R, tag="zrow", name="zrow")
            with nc.allow_low_precision(reason="f32r is bitwise fp32"):
                nc.vector.tensor_copy(
                    zrow[D_HEAD : D_HEAD + 1, :],
                    po[D_HEAD : D_HEAD + 1, :, :].rearrange("p a b -> p (a b)"),
                )
            for c in range(NCH):
                bc = psC.tile([P, 2, CH], FP32, tag="ps", name="bc")
                nc.tensor.matmul(
                    bc[0:D_HEAD, 0, :],
                    lhsT=ones_sb[D_HEAD : D_HEAD + 1, :],
                    rhs=zrow[D_HEAD : D_HEAD + 1, ts(c, CH)],
                    start=True,
                    stop=True,
                )
                rec = blk.tile([D_HEAD, CH], F32R, tag="rec", name="rec")
                with nc.allow_low_precision(reason="f32r is bitwise fp32"):
                    nc.vector.reciprocal(rec[:], bc[0:D_HEAD, 0, :])
                nc.vector.tensor_tensor(
                    o2s[h][:, c, :], po[0:D_HEAD, c, :], rec[:], MULT
                )

        def pw_block(ib):
            c, j = ib // NCH, ib % NCH
            pw = psC.tile([P, 2, CH], FP32, tag="ps", name="pw")
            for h in range(2):
                nc.tensor.matmul(
                    pw[:, 0, :],
                    lhsT=o2s[h][:, c, ts(j, P)],
                    rhs=Wo_sb[:, h, :],
                    start=(h == 0),
                    stop=(h == 1),
                )
            fin = blk.tile([P, D_MODEL], FP32, tag="fin", name="fin")
            if ib % 2 == 0:
                nc.vector.tensor_copy(fin[:], pw[:, 0, :])
            else:
                nc.scalar.copy(fin[:], pw[:, 0, :])
            nc.gpsimd.dma_start(out_partial[ts(ib, P), :], fin[:])

        # ---- prologue ----
        proj_groups(w_sbs["Wq"], xT_sb, evac_q)
        proj_groups(w_sbs["Wp"], posT_sb, evac_to(pT))
        # pos(h0) blocks 0..8 can start; interleave with remaining projections
        pre = [
            (lambda: proj_groups(w_sbs["Wk"], xT_sb, evac_to(kT))),
            (lambda: proj_v()),
        ]
        for ib in range(9):
            pos_block(0, ib)
            if pre:
                pre.pop(0)()

        # ---- main: per head, kt-outer within each q-half ----
        for h in range(2):
            po = psAcc.tile([D_HEAD + 1, NCH, CH], FP32, tag="po", name="po")
            for half in range(2):
                # lookahead pos blocks to interleave with this half's kt loop
                if h == 0 and half == 0:
                    look = [(0, b) for b in range(9, NB)]          # h0: 9..15
                elif h == 0 and half == 1:
                    look = [(1, b) for b in range(9)]              # h1: 0..8
                elif h == 1 and half == 0:
                    look = [(1, b) for b in range(9, NB)]          # h1: 9..15
                else:
                    look = []
                inter = [
                    (lambda hh, bb: (lambda: pos_block(hh, bb)))(hh, bb)
                    for hh, bb in look
                ]
                for kt in range(NK):
                    kt_step(h, half, kt, po, inter)
                for fn in inter:
                    fn()
            norm_head(h, po)

        for ib in range(NB):
            pw_block(ib)

    nc.finalize()
    return nc


# ---------------- host side ----------------

_NC_CACHE = {}


def _get_nc(S=S_FULL):
    if S not in _NC_CACHE:
        _NC_CACHE[S] = build_nc(S)
    return _NC_CACHE[S]


def make_in_maps(inputs, S=S_FULL, n_cores=8):
    bf16 = ml_dtypes.bfloat16
    x = np.asarray(inputs["x"], np.float32)
    pos = np.asarray(inputs["pos_embedding"], np.float32)
    Wq = np.asarray(inputs["Wq"], np.float32)
    bq = np.asarray(inputs["bq"], np.float32)
    Wk = np.asarray(inputs["Wk"], np.float32)
    Wv = np.asarray(inputs["Wv"], np.float32)
    Wp = np.asarray(inputs["Wp"], np.float32)
    u = np.asarray(inputs["u"], np.float32)
    v = np.asarray(inputs["v"], np.float32)
    Wo = np.asarray(inputs["Wo"], np.float32)

    xTb = [np.ascontiguousarray(x[b, :S].T).astype(bf16) for b in range(B_FULL)]
    posTb = [np.ascontiguousarray(pos[b, :S].T).astype(bf16) for b in range(B_FULL)]

    in_maps = []
    for c in range(n_cores):
        b = c // 4
        h0 = 2 * (c % 4)
        sl = slice(h0 * D_HEAD, (h0 + 2) * D_HEAD)
        u_eff = ((u[h0 : h0 + 2].reshape(-1) + bq[sl]) * ISQ).astype(np.float32)
        v_eff = ((v[h0 : h0 + 2].reshape(-1) + bq[sl]) * ISQ).astype(np.float32)
        in_maps.append(
            {
                "xT": xTb[b],
                "posT": posTb[b],
                "Wq": np.ascontiguousarray(Wq[:, sl]).astype(bf16),
                "Wk": np.ascontiguousarray(Wk[:, sl]).astype(bf16),
                "Wv": np.ascontiguousarray(Wv[:, sl]).astype(bf16),
                "Wp": np.ascontiguousarray(Wp[:, sl]).astype(bf16),
                "Wo": np.ascontiguousarray(Wo[sl, :]).astype(bf16),
                "ueff": u_eff.reshape(DH2, 1),
                "veff": v_eff.reshape(DH2, 1),
            }
        )
    return in_maps


def assemble(inputs, results, S=S_FULL):
    bv = np.asarray(inputs["bv"], np.float64)
    Wo = np.asarray(inputs["Wo"], np.float64)
    bo = np.asarray(inputs["bo"], np.float64)
    const = (bv @ Wo + bo).astype(np.float32)
    out = np.zeros((B_FULL, S, D_MODEL), np.float32)
    for c, res in enumerate(results):
        out[c // 4] += res["out_partial"]
    out += const[None, None, :]
    return out


def _run(inputs, trace=False, **kw):
    nc = _get_nc(S_FULL)
    in_maps = make_in_maps(inputs, S_FULL)
    res = run_bass_kernel_spmd(nc, in_maps, list(range(8)), trace=trace, **kw)
    out = assemble(inputs, res.results, S_FULL)
    return out, res


def kernel(**inputs) -> np.ndarray:
    out, _ = _run(inputs, trace=False)
    return out


# revision 20
# speedup vs baseline: 1.4909x; 1.2394x over previous
"""Trainium2 Bass kernel for Transformer-XL style relative-position MHSA.

Problem: nn_MultiHeadSelfAttention_14989435863450
  B=2, S=2048, D=512, H=8, dh=64, fp32 I/O.

Sharding (8 cores): core c -> batch b = c//4, head pair h0 = 2*(c%4).
Each core computes its 2 heads' attention and the partial output
projection; host sums 4 partials per batch and adds (bv @ Wo + bo).

Math folds (exact):
  - bq folds into u,v:  u_eff = (u + bq) / sqrt(D)
  - bk adds a per-query-row constant to scores -> cancels in softmax
  - bv contributes attn-weighted 1 * bv = bv -> host-side constant
  - 1/sqrt(D) folded into q at evacuation time

Design (v2, transposed attention, kt-outer):
  - All matmul operands bf16.
  - Scores built TRANSPOSED: sT[k, q] = kT.T @ qTu per 128-k tile; the
    attn matrix never needs a PE transpose before the attn@v matmul.
  - Rel-shift via DRAM buffer PB[S, S+1]; the shifted [q, k] view
    (flat[S + q*S + k]) is read back TRANSPOSED by the XBAR DMA in
    [1024, 128] panels (one per (h, q-half, k-tile)), alternating the
    two HWDGE rings (sync / scalar) so transposes pipeline.
  - Loop order kt-outer within a q-half: po accumulates into a 4-bank
    PSUM tile [65, 4*512] (slice per 512-q chunk); the vv stationary is
    reused across the half's two chunks.
  - Softmax denominator rides as a ones-column in vv: po row 64 is Z.
    Normalize per chunk: bcast = ones.T @ Z (PE), rec = 1/bcast (DVE,
    full-lane), o2 = po * rec.
  - PB writes and output writes ride the GPSIMD SWDGE ring, keeping the
    sync/scalar HWDGE rings free for the transposed reads.
"""

import math
from contextlib import ExitStack

import numpy as np
import ml_dtypes

import concourse.bass as bass
import concourse.bacc as bacc_mod
import concourse.mybir as mybir
import concourse.tile as tile
from concourse.bass import ts, ds
from concourse.bass_utils import run_bass_kernel_spmd

FP32 = mybir.dt.float32
F32R = mybir.dt.float32r
BF16 = mybir.dt.bfloat16

D_MODEL = 512
NUM_HEADS = 8
D_HEAD = 64
DH2 = 2 * D_HEAD
B_FULL = 2
S_FULL = 2048
P = 128
CH = 512
ISQ = 1.0 / math.sqrt(D_MODEL)

Exp = mybir.ActivationFunctionType.Exp
ADD = mybir.AluOpType.add
MULT = mybir.AluOpType.mult


def build_nc(S=S_FULL):
    nc = bacc_mod.Bacc()
    NB = S // P        # 16 q blocks
    NK = S // P        # 16 k tiles
    NCH = S // CH      # 4 chunks
    KD = D_MODEL // P  # 4
    HALF = S // 2      # 1024

    xT = nc.declare_dram_parameter("xT", [D_MODEL, S], BF16, isOutput=False)
    posT = nc.declare_dram_parameter("posT", [D_MODEL, S], BF16, isOutput=False)
    Wq = nc.declare_dram_parameter("Wq", [D_MODEL, DH2], BF16, isOutput=False)
    Wk = nc.declare_dram_parameter("Wk", [D_MODEL, DH2], BF16, isOutput=False)
    Wv = nc.declare_dram_parameter("Wv", [D_MODEL, DH2], BF16, isOutput=False)
    Wp = nc.declare_dram_parameter("Wp", [D_MODEL, DH2], BF16, isOutput=False)
    Wo = nc.declare_dram_parameter("Wo", [DH2, D_MODEL], BF16, isOutput=False)
    ueff = nc.declare_dram_parameter("ueff", [DH2, 1], FP32, isOutput=False)
    veff = nc.declare_dram_parameter("veff", [DH2, 1], FP32, isOutput=False)
    out_partial = nc.declare_dram_parameter("out_partial", [S, D_MODEL], FP32, isOutput=True)

    with ExitStack() as ctx:
        tc = ctx.enter_context(tile.TileContext(nc))
        consts = ctx.enter_context(tc.tile_pool(name="consts", bufs=1))
        blk = ctx.enter_context(tc.tile_pool(name="blk", bufs=3))
        spool = ctx.enter_context(tc.tile_pool(name="spool", bufs=5))
        dram = ctx.enter_context(tc.tile_pool(name="dram", bufs=1, space="DRAM"))
        # PSUM (8 banks): psAcc 1x[65,2,512] (2) + psC 3x[128,2,512] (6)
        psAcc = ctx.enter_context(tc.tile_pool(name="psAcc", bufs=1, space="PSUM"))
        psC = ctx.enter_context(tc.tile_pool(name="psC", bufs=3, space="PSUM"))

        # ---- load constants / inputs ----
        xT_sb = consts.tile([P, KD, S], BF16)
        nc.sync.dma_start(xT_sb[:], xT.rearrange("(o p) s -> p o s", p=P))
        posT_sb = consts.tile([P, KD, S], BF16)
        nc.sync.dma_start(posT_sb[:], posT.rearrange("(o p) s -> p o s", p=P))
        w_sbs = {}
        for nm, handle in (("Wq", Wq), ("Wp", Wp), ("Wk", Wk), ("Wv", Wv)):
            w_sb = consts.tile([P, KD, DH2], BF16, name=f"{nm}_sb")
            nc.sync.dma_start(w_sb[:], handle.rearrange("(o p) m -> p o m", p=P))
            w_sbs[nm] = w_sb
        Wo_sb = consts.tile([D_HEAD, 2, D_MODEL], BF16)
        nc.sync.dma_start(Wo_sb[:], Wo.rearrange("(h d) n -> d h n", h=2))
        ueff_sb = consts.tile([DH2, 1], FP32)
        nc.sync.dma_start(ueff_sb[:], ueff[:, :])
        veff_sb = consts.tile([DH2, 1], FP32)
        nc.sync.dma_start(veff_sb[:], veff[:, :])


        qTu = consts.tile([DH2, S], BF16)
        qTv = consts.tile([DH2, S], BF16)
        kT = consts.tile([DH2, S], BF16)
        pT = consts.tile([DH2, S], BF16)
        vv_aug = consts.tile([P, NK, 2, D_HEAD + 1], BF16)
        nc.vector.memset(vv_aug[:, :, :, D_HEAD : D_HEAD + 1], 1.0)

        def proj_groups(w_sb, src_sb, evac):
            for g in range(NCH // 2):
                pg = psC.tile([P, 2, CH], FP32, tag="ps", name="pg")
                for j in range(2):
                    chn = 2 * g + j
                    for kt in range(KD):
                        nc.tensor.matmul(
                            pg[:, j, :],
                            lhsT=w_sb[:, kt, :],
                            rhs=src_sb[:, kt, ts(chn, CH)],
                            start=(kt == 0),
                            stop=(kt == KD - 1),
                        )
                evac(g, pg)

        def evac_q(g, pg):
            sl = ds(g * 2 * CH, 2 * CH)
            pv = pg[:].rearrange("p a b -> p (a b)")
            nc.vector.tensor_scalar(qTu[:, sl], pv, ISQ, ueff_sb[:, 0:1], MULT, ADD)
            nc.vector.tensor_scalar(qTv[:, sl], pv, ISQ, veff_sb[:, 0:1], MULT, ADD)

        def evac_to(dst):
            def evac(g, pg):
                sl = ds(g * 2 * CH, 2 * CH)
                nc.scalar.copy(dst[:, sl], pg[:].rearrange("p a b -> p (a b)"))
            return evac

        def proj_v():
            for sg in range(NK // 2):
                pv = psC.tile([P, 2, CH], FP32, tag="ps", name="pv")
                for j in range(2):
                    st = 2 * sg + j
                    for kt in range(KD):
                        nc.tensor.matmul(
                            pv[:, j, 0:DH2],
                            lhsT=xT_sb[:, kt, ts(st, P)],
                            rhs=w_sbs["Wv"][:, kt, :],
                            start=(kt == 0),
                            stop=(kt == KD - 1),
                        )
                for j in range(2):
                    src = pv[:, j, 0:DH2].rearrange("p (h d) -> p h d", h=2)
                    nc.vector.tensor_copy(vv_aug[:, 2 * sg + j, :, 0:D_HEAD], src)

        PB = [dram.tile([S, S + 1], BF16, name=f"pb{h}") for h in range(2)]

        def pos_block(h, ib):
            """pos scores (orientation A) for q rows [128*ib, +128) -> PB[h]."""
            pe = blk.tile([P, S + 1], BF16, tag="posext", name="pe")
            nc.vector.memset(pe[:, 0:1], 0.0)
            for g in range(NCH // 2):
                pp = psC.tile([P, 2, CH], FP32, tag="ps", name="pp")
                for j in range(2):
                    chn = 2 * g + j
                    nc.tensor.matmul(
                        pp[:, j, :],
                        lhsT=qTv[ds(h * D_HEAD, D_HEAD), ts(ib, P)],
                        rhs=pT[ds(h * D_HEAD, D_HEAD), ts(chn, CH)],
                        start=True,
                        stop=True,
                    )
                dst = pe[:, ds(1 + g * 2 * CH, 2 * CH)]
                src = pp[:].rearrange("p a b -> p (a b)")
                if (ib + g) % 2 == 0:
                    nc.scalar.copy(dst, src)
                else:
                    nc.vector.tensor_copy(dst, src)
            nc.gpsimd.dma_start(PB[h][ts(ib, P), :], pe[:])

        # unnormalized attn@v results (row 64 = softmax denominator Z)
        o2u = {}
        rz = {}
        for h in range(2):
            o2u[h] = blk.tile([D_HEAD + 1, NCH, CH], BF16, tag=f"o2_{h}", name="o2u")
            rz[h] = blk.tile([P, NB], FP32, tag=f"rz_{h}", name="rz")
        zd = dram.tile([2, S], BF16, name="zd")

        def kt_step(h, half, kt, po, interleave):
            """one k-tile for q rows [half*1024, +1024): read spT, scores,
            exp, accumulate attn@v into po."""
            sp = spool.tile([P, 2, CH], BF16, tag="spos", name="sp")
            flat = PB[h].flatten()
            qview = flat[ds(S + half * HALF * S, HALF * S)].rearrange(
                "(q k) -> q k", k=S
            )
            eng = nc.sync if kt % 2 == 0 else nc.scalar
            eng.dma_start(sp[:].rearrange("p a b -> p (a b)"),
                          qview[:, ts(kt, P)], transpose=True)
            ps = psC.tile([P, 2, CH], FP32, tag="ps", name="ps")
            for j in range(2):
                c = 2 * half + j
                nc.tensor.matmul(
                    ps[:, j, :],
                    lhsT=kT[ds(h * D_HEAD, D_HEAD), ts(kt, P)],
                    rhs=qTu[ds(h * D_HEAD, D_HEAD), ts(c, CH)],
                    start=True,
                    stop=True,
                )
            if interleave:
                interleave.pop(0)()
            sc = blk.tile([P, 2, CH], BF16, tag="sc", name="sc")
            nc.vector.tensor_tensor(sc[:], ps[:], sp[:], ADD)
            et = blk.tile([P, 2, CH], BF16, tag="et", name="et")
            nc.scalar.activation(et[:], sc[:], Exp)
            for j in range(2):
                nc.tensor.matmul(
                    po[:, j, :],
                    lhsT=vv_aug[:, kt, h, :],
                    rhs=et[:, j, :],
                    start=(kt == 0),
                    stop=(kt == NK - 1),
                )

        def evac_half(h, half, po):
            """po [65, 2, 512] -> o2u[h] chunks of this half (incl. Z row)."""
            dst = o2u[h][:, ts(half, 2), :]
            if (h + half) % 2 == 0:
                nc.vector.tensor_copy(dst, po[:])
            else:
                nc.scalar.copy(dst, po[:])

        def finish_head(h):
            """Z row -> DRAM -> xbar-transposed [128, 16] -> rz = 1/Z."""
            nc.gpsimd.dma_start(
                zd[h : h + 1, :],
                o2u[h][D_HEAD : D_HEAD + 1, :, :].rearrange("p a b -> p (a b)"),
            )
            zview = zd.flatten()[ds(h * S, S)].rearrange("(a b) -> a b", b=P)
            rzt = blk.tile([P, NB], BF16, tag=f"rzt_{h}", name="rzt")
            nc.sync.dma_start(rzt[:], zview, transpose=True)
            nc.vector.reciprocal(rz[h][:], rzt[:])

        def pw_block(ib):
            c, j = ib // NCH, ib % NCH
            pw = psC.tile([P, 2, CH], FP32, tag="ps", name="pw")
            for h in range(2):
                nc.tensor.matmul(
                    pw[:, h, :],
                    lhsT=o2u[h][0:D_HEAD, c, ts(j, P)],
                    rhs=Wo_sb[:, h, :],
                    start=True,
                    stop=True,
                )
            t1 = blk.tile([P, D_MODEL], FP32, tag="t1", name="t1")
            nc.scalar.mul(t1[:], pw[:, 1, :], rz[1][:, ib : ib + 1])
            fin = blk.tile([P, D_MODEL], FP32, tag="fin", name="fin")
            nc.vector.scalar_tensor_tensor(
                fin[:], pw[:, 0, :], rz[0][:, ib : ib + 1], t1[:], MULT, ADD
            )
            nc.gpsimd.dma_start(out_partial[ts(ib, P), :], fin[:])

        # ---- prologue ----
        proj_groups(w_sbs["Wq"], xT_sb, evac_q)
        proj_groups(w_sbs["Wp"], posT_sb, evac_to(pT))
        # pos(h0) blocks 0..8 can start; interleave with remaining projections
        pre = [
            (lambda: proj_groups(w_sbs["Wk"], xT_sb, evac_to(kT))),
            (lambda: proj_v()),
        ]
        for ib in range(9):
            pos_block(0, ib)
            if pre:
                pre.pop(0)()

        # ---- main: per head, kt-outer within each q-half ----
        for h in range(2):
            for half in range(2):
                po = psAcc.tile([D_HEAD + 1, 2, CH], FP32, tag="po", name="po")
                # lookahead pos blocks to interleave with this half's kt loop
                if h == 0 and half == 0:
                    look = [(0, b) for b in range(9, NB)]          # h0: 9..15
                elif h == 0 and half == 1:
                    look = [(1, b) for b in range(9)]              # h1: 0..8
                elif h == 1 and half == 0:
                    look = [(1, b) for b in range(9, NB)]          # h1: 9..15
                else:
                    look = []
                inter = [
                    (lambda hh, bb: (lambda: pos_block(hh, bb)))(hh, bb)
                    for hh, bb in look
                ]
                for kt in range(NK):
                    kt_step(h, half, kt, po, inter)
                for fn in inter:
                    fn()
                evac_half(h, half, po)
            finish_head(h)

        for ib in range(NB):
            pw_block(ib)

    nc.finalize()
    return nc


# ---------------- host side ----------------

_NC_CACHE = {}


def _get_nc(S=S_FULL):
    if S not in _NC_CACHE:
        _NC_CACHE[S] = build_nc(S)
    return _NC_CACHE[S]


def make_in_maps(inputs, S=S_FULL, n_cores=8):
    bf16 = ml_dtypes.bfloat16
    x = np.asarray(inputs["x"], np.float32)
    pos = np.asarray(inputs["pos_embedding"], np.float32)
    Wq = np.asarray(inputs["Wq"], np.float32)
    bq = np.asarray(inputs["bq"], np.float32)
    Wk = np.asarray(inputs["Wk"], np.float32)
    Wv = np.asarray(inputs["Wv"], np.float32)
    Wp = np.asarray(inputs["Wp"], np.float32)
    u = np.asarray(inputs["u"], np.float32)
    v = np.asarray(inputs["v"], np.float32)
    Wo = np.asarray(inputs["Wo"], np.float32)

    xTb = [np.ascontiguousarray(x[b, :S].T).astype(bf16) for b in range(B_FULL)]
    posTb = [np.ascontiguousarray(pos[b, :S].T).astype(bf16) for b in range(B_FULL)]

    in_maps = []
    for c in range(n_cores):
        b = c // 4
        h0 = 2 * (c % 4)
        sl = slice(h0 * D_HEAD, (h0 + 2) * D_HEAD)
        u_eff = ((u[h0 : h0 + 2].reshape(-1) + bq[sl]) * ISQ).astype(np.float32)
        v_eff = ((v[h0 : h0 + 2].reshape(-1) + bq[sl]) * ISQ).astype(np.float32)
        in_maps.append(
            {
                "xT": xTb[b],
                "posT": posTb[b],
                "Wq": np.ascontiguousarray(Wq[:, sl]).astype(bf16),
                "Wk": np.ascontiguousarray(Wk[:, sl]).astype(bf16),
                "Wv": np.ascontiguousarray(Wv[:, sl]).astype(bf16),
                "Wp": np.ascontiguousarray(Wp[:, sl]).astype(bf16),
                "Wo": np.ascontiguousarray(Wo[sl, :]).astype(bf16),
                "ueff": u_eff.reshape(DH2, 1),
                "veff": v_eff.reshape(DH2, 1),
            }
        )
    return in_maps


def assemble(inputs, results, S=S_FULL):
    bv = np.asarray(inputs["bv"], np.float64)
    Wo = np.asarray(inputs["Wo"], np.float64)
    bo = np.asarray(inputs["bo"], np.float64)
    const = (bv @ Wo + bo).astype(np.float32)
    out = np.zeros((B_FULL, S, D_MODEL), np.float32)
    for c, res in enumerate(results):
        out[c // 4] += res["out_partial"]
    out += const[None, None, :]
    return out


def _run(inputs, trace=False, **kw):
    nc = _get_nc(S_FULL)
    in_maps = make_in_maps(inputs, S_FULL)
    res = run_bass_kernel_spmd(nc, in_maps, list(range(8)), trace=trace, **kw)
    out = assemble(inputs, res.results, S_FULL)
    return out, res


def kernel(**inputs) -> np.ndarray:
    out, _ = _run(inputs, trace=False)
    return out


# revision 22
# speedup vs baseline: 1.5491x; 1.0390x over previous
"""Trainium2 Bass kernel for Transformer-XL style relative-position MHSA.

Problem: nn_MultiHeadSelfAttention_14989435863450
  B=2, S=2048, D=512, H=8, dh=64, fp32 I/O.

Sharding (8 cores): core c -> batch b = c//4, head pair h0 = 2*(c%4).
Each core computes its 2 heads' attention and the partial output
projection; host sums 4 partials per batch and adds (bv @ Wo + bo).

Math folds (exact):
  - bq folds into u,v:  u_eff = (u + bq) / sqrt(D)
  - bk adds a per-query-row constant to scores -> cancels in softmax
  - bv contributes attn-weighted 1 * bv = bv -> host-side constant
  - 1/sqrt(D) folded into q at evacuation time

Design (v2, transposed attention, kt-outer):
  - All matmul operands bf16.
  - Scores built TRANSPOSED: sT[k, q] = kT.T @ qTu per 128-k tile; the
    attn matrix never needs a PE transpose before the attn@v matmul.
  - Rel-shift via DRAM buffer PB[S, S+1]; the shifted [q, k] view
    (flat[S + q*S + k]) is read back TRANSPOSED by the XBAR DMA in
    [1024, 128] panels (one per (h, q-half, k-tile)), alternating the
    two HWDGE rings (sync / scalar) so transposes pipeline.
  - Loop order kt-outer within a q-half: po accumulates into a 4-bank
    PSUM tile [65, 4*512] (slice per 512-q chunk); the vv stationary is
    reused across the half's two chunks.
  - Softmax denominator rides as a ones-column in vv: po row 64 is Z.
    Normalize per chunk: bcast = ones.T @ Z (PE), rec = 1/bcast (DVE,
    full-lane), o2 = po * rec.
  - PB writes and output writes ride the GPSIMD SWDGE ring, keeping the
    sync/scalar HWDGE rings free for the transposed reads.
"""

import math
from contextlib import ExitStack

import numpy as np
import ml_dtypes

import concourse.bass as bass
import concourse.bacc as bacc_mod
import concourse.mybir as mybir
import concourse.tile as tile
from concourse.bass import ts, ds
from concourse.bass_utils import run_bass_kernel_spmd

FP32 = mybir.dt.float32
F32R = mybir.dt.float32r
BF16 = mybir.dt.bfloat16

D_MODEL = 512
NUM_HEADS = 8
D_HEAD = 64
DH2 = 2 * D_HEAD
B_FULL = 2
S_FULL = 2048
P = 128
CH = 512
ISQ = 1.0 / math.sqrt(D_MODEL)

Exp = mybir.ActivationFunctionType.Exp
ADD = mybir.AluOpType.add
MULT = mybir.AluOpType.mult


def build_nc(S=S_FULL):
    nc = bacc_mod.Bacc()
    NB = S // P        # 16 q blocks
    NK = S // P        # 16 k tiles
    NCH = S // CH      # 4 chunks
    KD = D_MODEL // P  # 4
    HALF = S // 2      # 1024

    xT = nc.declare_dram_parameter("xT", [D_MODEL, S], BF16, isOutput=False)
    posT = nc.declare_dram_parameter("posT", [D_MODEL, S], BF16, isOutput=False)
    Wq = nc.declare_dram_parameter("Wq", [D_MODEL, DH2], BF16, isOutput=False)
    Wk = nc.declare_dram_parameter("Wk", [D_MODEL, DH2], BF16, isOutput=False)
    Wv = nc.declare_dram_parameter("Wv", [D_MODEL, DH2], BF16, isOutput=False)
    Wp = nc.declare_dram_parameter("Wp", [D_MODEL, DH2], BF16, isOutput=False)
    Wo = nc.declare_dram_parameter("Wo", [DH2, D_MODEL], BF16, isOutput=False)
    ueff = nc.declare_dram_parameter("ueff", [DH2, 1], FP32, isOutput=False)
    veff = nc.declare_dram_parameter("veff", [DH2, 1], FP32, isOutput=False)
    out_partial = nc.declare_dram_parameter("out_partial", [S, D_MODEL], FP32, isOutput=True)

    with ExitStack() as ctx:
        tc = ctx.enter_context(tile.TileContext(nc))
        consts = ctx.enter_context(tc.tile_pool(name="consts", bufs=1))
        blk = ctx.enter_context(tc.tile_pool(name="blk", bufs=3))
        spool = ctx.enter_context(tc.tile_pool(name="spool", bufs=5))
        dram = ctx.enter_context(tc.tile_pool(name="dram", bufs=1, space="DRAM"))
        # PSUM (8 banks): psAcc 1x[65,2,512] (2) + psC 3x[128,2,512] (6)
        psAcc = ctx.enter_context(tc.tile_pool(name="psAcc", bufs=1, space="PSUM"))
        psC = ctx.enter_context(tc.tile_pool(name="psC", bufs=3, space="PSUM"))

        # ---- load constants / inputs ----
        xT_sb = consts.tile([P, KD, S], BF16)
        nc.sync.dma_start(xT_sb[:], xT.rearrange("(o p) s -> p o s", p=P))
        posT_sb = consts.tile([P, KD, S], BF16)
        nc.sync.dma_start(posT_sb[:], posT.rearrange("(o p) s -> p o s", p=P))
        w_sbs = {}
        for nm, handle in (("Wq", Wq), ("Wp", Wp), ("Wk", Wk), ("Wv", Wv)):
            w_sb = consts.tile([P, KD, DH2], BF16, name=f"{nm}_sb")
            nc.sync.dma_start(w_sb[:], handle.rearrange("(o p) m -> p o m", p=P))
            w_sbs[nm] = w_sb
        Wo_sb = consts.tile([D_HEAD, 2, D_MODEL], BF16)
        nc.sync.dma_start(Wo_sb[:], Wo.rearrange("(h d) n -> d h n", h=2))
        ueff_sb = consts.tile([DH2, 1], FP32)
        nc.sync.dma_start(ueff_sb[:], ueff[:, :])
        veff_sb = consts.tile([DH2, 1], FP32)
        nc.sync.dma_start(veff_sb[:], veff[:, :])


        qTu = consts.tile([DH2, S], BF16)
        qTv = consts.tile([DH2, S], BF16)
        kT = consts.tile([DH2, S], BF16)
        pT = consts.tile([DH2, S], BF16)
        vv_aug = consts.tile([P, NK, 2, D_HEAD + 1], BF16)
        nc.vector.memset(vv_aug[:, :, :, D_HEAD : D_HEAD + 1], 1.0)

        def proj_groups(w_sb, src_sb, evac):
            for g in range(NCH // 2):
                pg = psC.tile([P, 2, CH], FP32, tag="ps", name="pg")
                for j in range(2):
                    chn = 2 * g + j
                    for kt in range(KD):
                        nc.tensor.matmul(
                            pg[:, j, :],
                            lhsT=w_sb[:, kt, :],
                            rhs=src_sb[:, kt, ts(chn, CH)],
                            start=(kt == 0),
                            stop=(kt == KD - 1),
                        )
                evac(g, pg)

        def evac_q(g, pg):
            sl = ds(g * 2 * CH, 2 * CH)
            pv = pg[:].rearrange("p a b -> p (a b)")
            nc.vector.tensor_scalar(qTu[:, sl], pv, ISQ, ueff_sb[:, 0:1], MULT, ADD)
            nc.vector.tensor_scalar(qTv[:, sl], pv, ISQ, veff_sb[:, 0:1], MULT, ADD)

        def evac_to(dst):
            def evac(g, pg):
                sl = ds(g * 2 * CH, 2 * CH)
                nc.scalar.copy(dst[:, sl], pg[:].rearrange("p a b -> p (a b)"))
            return evac

        def proj_v():
            for sg in range(NK // 2):
                pv = psC.tile([P, 2, CH], FP32, tag="ps", name="pv")
                for j in range(2):
                    st = 2 * sg + j
                    for kt in range(KD):
                        nc.tensor.matmul(
                            pv[:, j, 0:DH2],
                            lhsT=xT_sb[:, kt, ts(st, P)],
                            rhs=w_sbs["Wv"][:, kt, :],
                            start=(kt == 0),
                            stop=(kt == KD - 1),
                        )
                for j in range(2):
                    src = pv[:, j, 0:DH2].rearrange("p (h d) -> p h d", h=2)
                    nc.vector.tensor_copy(vv_aug[:, 2 * sg + j, :, 0:D_HEAD], src)

        PB = [dram.tile([S, S + 1], BF16, name=f"pb{h}") for h in range(2)]

        def pos_block(h, ib):
            """pos scores (orientation A) for q rows [128*ib, +128) -> PB[h]."""
            pe = blk.tile([P, S + 1], BF16, tag="posext", name="pe")
            nc.vector.memset(pe[:, 0:1], 0.0)
            for g in range(NCH // 2):
                pp = psC.tile([P, 2, CH], FP32, tag="ps", name="pp")
                for j in range(2):
                    chn = 2 * g + j
                    nc.tensor.matmul(
                        pp[:, j, :],
                        lhsT=qTv[ds(h * D_HEAD, D_HEAD), ts(ib, P)],
                        rhs=pT[ds(h * D_HEAD, D_HEAD), ts(chn, CH)],
                        start=True,
                        stop=True,
                    )
                dst = pe[:, ds(1 + g * 2 * CH, 2 * CH)]
                src = pp[:].rearrange("p a b -> p (a b)")
                if (ib + g) % 2 == 0:
                    nc.scalar.copy(dst, src)
                else:
                    nc.vector.tensor_copy(dst, src)
            nc.gpsimd.dma_start(PB[h][ts(ib, P), :], pe[:])

        # unnormalized attn@v results (row 64 = softmax denominator Z)
        o2u = {}
        rz = {}
        for h in range(2):
            o2u[h] = blk.tile([D_HEAD + 1, NCH, CH], BF16, tag=f"o2_{h}", name="o2u")
            rz[h] = blk.tile([P, NB], FP32, tag=f"rz_{h}", name="rz")
        zd = dram.tile([2, S], BF16, name="zd")

        def issue_read(h, half, kt):
            """prefetch the shifted+transposed pos panel for (h, half, kt)."""
            sp = spool.tile([P, 2, CH], BF16, tag="spos", name="sp")
            flat = PB[h].flatten()
            qview = flat[ds(S + half * HALF * S, HALF * S)].rearrange(
                "(q k) -> q k", k=S
            )
            eng = nc.sync if kt % 2 == 0 else nc.scalar
            eng.dma_start(sp[:].rearrange("p a b -> p (a b)"),
                          qview[:, ts(kt, P)], transpose=True)
            return sp

        def po_step(h, kt, po, et):
            for j in range(2):
                nc.tensor.matmul(
                    po[:, j, :],
                    lhsT=vv_aug[:, kt, h, :],
                    rhs=et[:, j, :],
                    start=(kt == 0),
                    stop=(kt == NK - 1),
                )

        def kt_step(h, half, kt, po, sp, prev, interleave):
            """content scores + exp for k-tile kt; attn@v for k-tile kt-1
            (delayed one step so exp(kt-1) is off the PE critical path)."""
            ps = psC.tile([P, 2, CH], FP32, tag="ps", name="ps")
            for j in range(2):
                c = 2 * half + j
                nc.tensor.matmul(
                    ps[:, j, :],
                    lhsT=kT[ds(h * D_HEAD, D_HEAD), ts(kt, P)],
                    rhs=qTu[ds(h * D_HEAD, D_HEAD), ts(c, CH)],
                    start=True,
                    stop=True,
                )
            if interleave:
                interleave.pop(0)()
            if prev is not None:
                po_step(h, prev[0], po, prev[1])
            sc = blk.tile([P, 2, CH], BF16, tag="sc", name="sc")
            nc.vector.tensor_tensor(sc[:], ps[:], sp[:], ADD)
            et = blk.tile([P, 2, CH], BF16, tag="et", name="et")
            nc.scalar.activation(et[:], sc[:], Exp)
            return (kt, et)

        def evac_half(h, half, po):
            """po [65, 2, 512] -> o2u[h] chunks of this half (incl. Z row)."""
            dst = o2u[h][:, ts(half, 2), :]
            if (h + half) % 2 == 0:
                nc.vector.tensor_copy(dst, po[:])
            else:
                nc.scalar.copy(dst, po[:])

        def finish_head(h):
            """Z row -> DRAM -> xbar-transposed [128, 16] -> rz = 1/Z."""
            nc.gpsimd.dma_start(
                zd[h : h + 1, :],
                o2u[h][D_HEAD : D_HEAD + 1, :, :].rearrange("p a b -> p (a b)"),
            )
            zview = zd.flatten()[ds(h * S, S)].rearrange("(a b) -> a b", b=P)
            rzt = blk.tile([P, NB], BF16, tag=f"rzt_{h}", name="rzt")
            nc.sync.dma_start(rzt[:], zview, transpose=True)
            nc.vector.reciprocal(rz[h][:], rzt[:])

        def pw_block(ib):
            c, j = ib // NCH, ib % NCH
            pw = psC.tile([P, 2, CH], FP32, tag="ps", name="pw")
            for h in range(2):
                nc.tensor.matmul(
                    pw[:, h, :],
                    lhsT=o2u[h][0:D_HEAD, c, ts(j, P)],
                    rhs=Wo_sb[:, h, :],
                    start=True,
                    stop=True,
                )
            t1 = blk.tile([P, D_MODEL], FP32, tag="t1", name="t1")
            nc.scalar.mul(t1[:], pw[:, 1, :], rz[1][:, ib : ib + 1])
            fin = blk.tile([P, D_MODEL], FP32, tag="fin", name="fin")
            nc.vector.scalar_tensor_tensor(
                fin[:], pw[:, 0, :], rz[0][:, ib : ib + 1], t1[:], MULT, ADD
            )
            nc.gpsimd.dma_start(out_partial[ts(ib, P), :], fin[:])

        # ---- prologue ----
        proj_groups(w_sbs["Wq"], xT_sb, evac_q)
        proj_groups(w_sbs["Wp"], posT_sb, evac_to(pT))
        # pos(h0) blocks 0..8 can start; interleave with remaining projections
        pre = [
            (lambda: proj_groups(w_sbs["Wk"], xT_sb, evac_to(kT))),
            (lambda: proj_v()),
        ]
        for ib in range(9):
            pos_block(0, ib)
            if pre:
                pre.pop(0)()

        # ---- main: per head, kt-outer within each q-half ----
        for h in range(2):
            for half in range(2):
                po = psAcc.tile([D_HEAD + 1, 2, CH], FP32, tag="po", name="po")
                # lookahead pos blocks to interleave with this half's kt loop
                if h == 0 and half == 0:
                    look = [(0, b) for b in range(9, NB)]          # h0: 9..15
                elif h == 0 and half == 1:
                    look = [(1, b) for b in range(9)]              # h1: 0..8
                elif h == 1 and half == 0:
                    look = [(1, b) for b in range(9, NB)]          # h1: 9..15
                else:
                    look = []
                inter = [
                    (lambda hh, bb: (lambda: pos_block(hh, bb)))(hh, bb)
                    for hh, bb in look
                ]
                sps = {0: issue_read(h, half, 0), 1: issue_read(h, half, 1)}
                prev = None
                for kt in range(NK):
                    if kt + 2 < NK:
                        sps[kt + 2] = issue_read(h, half, kt + 2)
                    prev = kt_step(h, half, kt, po, sps.pop(kt), prev, inter)
                po_step(h, prev[0], po, prev[1])
                for fn in inter:
                    fn()
                evac_half(h, half, po)
            finish_head(h)

        for ib in range(NB):
            pw_block(ib)

    nc.finalize()
    return nc


# ---------------- host side ----------------

_NC_CACHE = {}


def _get_nc(S=S_FULL):
    if S not in _NC_CACHE:
        _NC_CACHE[S] = build_nc(S)
    return _NC_CACHE[S]


def make_in_maps(inputs, S=S_FULL, n_cores=8):
    bf16 = ml_dtypes.bfloat16
    x = np.asarray(inputs["x"], np.float32)
    pos = np.asarray(inputs["pos_embedding"], np.float32)
    Wq = np.asarray(inputs["Wq"], np.float32)
    bq = np.asarray(inputs["bq"], np.float32)
    Wk = np.asarray(inputs["Wk"], np.float32)
    Wv = np.asarray(inputs["Wv"], np.float32)
    Wp = np.asarray(inputs["Wp"], np.float32)
    u = np.asarray(inputs["u"], np.float32)
    v = np.asarray(inputs["v"], np.float32)
    Wo = np.asarray(inputs["Wo"], np.float32)

    xTb = [np.ascontiguousarray(x[b, :S].T).astype(bf16) for b in range(B_FULL)]
    posTb = [np.ascontiguousarray(pos[b, :S].T).astype(bf16) for b in range(B_FULL)]

    in_maps = []
    for c in range(n_cores):
        b = c // 4
        h0 = 2 * (c % 4)
        sl = slice(h0 * D_HEAD, (h0 + 2) * D_HEAD)
        u_eff = ((u[h0 : h0 + 2].reshape(-1) + bq[sl]) * ISQ).astype(np.float32)
        v_eff = ((v[h0 : h0 + 2].reshape(-1) + bq[sl]) * ISQ).astype(np.float32)
        in_maps.append(
            {
                "xT": xTb[b],
                "posT": posTb[b],
                "Wq": np.ascontiguousarray(Wq[:, sl]).astype(bf16),
                "Wk": np.ascontiguousarray(Wk[:, sl]).astype(bf16),
                "Wv": np.ascontiguousarray(Wv[:, sl]).astype(bf16),
                "Wp": np.ascontiguousarray(Wp[:, sl]).astype(bf16),
                "Wo": np.ascontiguousarray(Wo[sl, :]).astype(bf16),
                "ueff": u_eff.reshape(DH2, 1),
                "veff": v_eff.reshape(DH2, 1),
            }
        )
    return in_maps


def assemble(inputs, results, S=S_FULL):
    bv = np.asarray(inputs["bv"], np.float64)
    Wo = np.asarray(inputs["Wo"], np.float64)
    bo = np.asarray(inputs["bo"], np.float64)
    const = (bv @ Wo + bo).astype(np.float32)
    out = np.zeros((B_FULL, S, D_MODEL), np.float32)
    for c, res in enumerate(results):
        out[c // 4] += res["out_partial"]
    out += const[None, None, :]
    return out


def _run(inputs, trace=False, **kw):
    nc = _get_nc(S_FULL)
    in_maps = make_in_maps(inputs, S_FULL)
    res = run_bass_kernel_spmd(nc, in_maps, list(range(8)), trace=trace, **kw)
    out = assemble(inputs, res.results, S_FULL)
    return out, res


def kernel(**inputs) -> np.ndarray:
    out, _ = _run(inputs, trace=False)
    return out
